# revision 44
# baseline (speedup 1.0000x reference)
import sys

sys.path.insert(0, "/opt/trn_rl_repo")

import numpy as np

N = 1024
T = 128
NCORES = 8

_BUILD_CACHE = {}
LAST_RESULT = None


def _units():
    # Units aligned to absolute 512-col windows.
    # q < 16: two units [32q, 512) and [512, 1024); q >= 16: one [32q, 1024).
    units = []
    for q in range(32):
        if q < 16:
            units.append((q, 32 * q, 512, True))
            units.append((q, 512, 1024, False))
        else:
            units.append((q, 32 * q, 1024, True))
    return units


def _groups():
    # Each group: 4 units sharing one PSUM bank (partition slots 0,32,64,96),
    # one 512-col window. Returns list of (base_col, [unit_idx x4]).
    gs = []
    for k in range(4):
        gs.append((0, [2 * q for q in range(4 * k, 4 * k + 4)]))
        gs.append((512, [2 * q + 1 for q in range(4 * k, 4 * k + 4)]))
    for j in range(4):
        gs.append((512, [16 + q for q in range(16 + 4 * j, 16 + 4 * j + 4)]))
    return gs


def _u_last(q):
    return 2 * q + 1 if q < 16 else 16 + q


CR = 6  # C/h1-group ring depth (in q-groups)
EARLY_TILE_UNITS = {36: 0, 38: 1, 40: 2, 42: 3}  # unit -> early tile idx


def _build(b3v):
    import concourse.bass as bass
    from concourse import mybir

    F32 = mybir.dt.float32
    F16 = mybir.dt.float16
    BF16 = mybir.dt.bfloat16
    Alu = mybir.AluOpType
    Act = mybir.ActivationFunctionType

    units = _units()
    U = len(units)  # 48
    groups = _groups()
    G = len(groups)  # 12
    ug = {}
    for g, (base, us) in enumerate(groups):
        for s, u in enumerate(us):
            ug[u] = (g, s)
    g_last_u = [us[-1] for (base, us) in groups]
    # groups whose diag fixes run on gpsimd (mask-evac covers 10, 11)
    gp_fix_groups = [
        g for g in range(G - 2) if any(units[u][3] for u in groups[g][1])
    ]

    nc = bass.Bass("TRN2", target_bir_lowering=False, debug=False, num_devices=8)

    def cwidth(q):
        return 4 * (N - 32 * q)

    # ---- inputs ----
    # C_q = pre-biased tanh inputs for group q: [128, 4*(N-32q)] bf16,
    # laid out as 4 consecutive blocks (one per t=4q+r).
    C_d = [
        nc.dram_tensor(f"c{q}", (128, cwidth(q)), BF16, kind="ExternalInput")
        for q in range(32)
    ]
    w2t_d = nc.dram_tensor("w2t", (128, 32), BF16, kind="ExternalInput")
    w3b_d = nc.dram_tensor("w3b", (128, 4), BF16, kind="ExternalInput")
    b2t_d = nc.dram_tensor("b2t", (128, 1), F32, kind="ExternalInput")
    mg_d = [
        nc.dram_tensor(f"mg{i}", (128, 512), BF16, kind="ExternalInput")
        for i in range(2)
    ]
    dg_d = [
        nc.dram_tensor(f"dg{i}", (128, 512), BF16, kind="ExternalInput")
        for i in range(2)
    ]
    m32_d = nc.dram_tensor("m32", (128, 32), BF16, kind="ExternalInput")
    d32_d = nc.dram_tensor("d32", (128, 32), BF16, kind="ExternalInput")
    # outputs: oe{m} = G[128m:128m+128, 0:512] (m=0..3),
    # ot{m} = G[128m:128m+128, 512:1024] (m=0..7); f16 partials.
    oute_d = [
        nc.dram_tensor(f"oe{m}", (128, 512), F16, kind="ExternalOutput")
        for m in range(4)
    ]
    outt_d = [
        nc.dram_tensor(f"ot{m}", (128, 512), F16, kind="ExternalOutput")
        for m in range(8)
    ]

    # ---- sbuf ----
    C_s = [
        nc.alloc_sbuf_tensor(f"C_s{i}", [128, 4 * N], BF16) for i in range(CR)
    ]
    hg_s = [
        nc.alloc_sbuf_tensor(f"hg_s{i}", [128, 4 * N], BF16) for i in range(CR)
    ]
    w2t_s = nc.alloc_sbuf_tensor("w2t_s", [128, 32], BF16)
    w3b_s = nc.alloc_sbuf_tensor("w3b_s", [128, 4], BF16)
    b2t_s = nc.alloc_sbuf_tensor("b2t_s", [128, 1], F32)
    mg_s = [
        nc.alloc_sbuf_tensor(f"mg_s{i}", [128, 512], BF16) for i in range(2)
    ]
    dg_s = [
        nc.alloc_sbuf_tensor(f"dg_s{i}", [128, 512], BF16) for i in range(2)
    ]
    m32_s = nc.alloc_sbuf_tensor("m32_s", [128, 32], BF16)
    d32_s = nc.alloc_sbuf_tensor("d32_s", [128, 32], BF16)
    t32_s = nc.alloc_sbuf_tensor("t32_s", [128, 32], BF16)
    h2_s = [nc.alloc_sbuf_tensor(f"h2_{i}", [128, 512], BF16) for i in range(4)]
    stage_s = [
        nc.alloc_sbuf_tensor(f"stage{i}", [128, 512], BF16) for i in range(4)
    ]
    K_s = nc.alloc_sbuf_tensor("K_s", [128, N], BF16)
    o_s = [nc.alloc_sbuf_tensor(f"o_s{i}", [128, 512], F16) for i in range(12)]

    # ---- psum: 4 ph + 2 pv + 2 pk = 8 banks ----
    ph_ps = [nc.alloc_psum_tensor(f"ph{i}", [128, 512], F32) for i in range(4)]
    pv_ps = [nc.alloc_psum_tensor(f"pv{i}", [128, 512], F32) for i in range(2)]
    pk_ps = [nc.alloc_psum_tensor(f"pk{i}", [128, 512], F32) for i in range(2)]

    NPRE = CR + 8  # preloads: b2t, C1..C{CR-1}, 8 smalls (C0 via scalar q)

    def dmain_c(q):
        # dma_in value once C_q has landed (q >= 1; C0 uses the c0s sem)
        if q < CR:
            return 16 * (q + 1)
        return 16 * (NPRE + (q - CR) + 1)

    def mk_waiter(engine):
        seen = {}

        def w(sem, val):
            if seen.get(id(sem), 0) < val:
                engine.wait_ge(sem, val)
                seen[id(sem)] = val

        return w

    with (
        nc.Block() as block,
        nc.semaphore("dma_in") as dma_in,
        nc.semaphore("mset") as mset,
        nc.semaphore("c0s") as c0s,
        nc.semaphore("acts") as acts,
        nc.semaphore("pes") as pes,
        nc.semaphore("pek") as pek,
        nc.semaphore("relu_s") as relu_s,
        nc.semaphore("vex") as vex,
        nc.semaphore("evacv") as evacv,
        nc.semaphore("evace") as evace,
        nc.semaphore("kg") as kg,
        nc.semaphore("ks") as ks,
        nc.semaphore("odma") as odma,
    ):
        def sp_group_dmas(sync, w, g):
            w(vex, g + 1)
            base, us = groups[g]
            for s in (0, 1):
                u = us[s]
                q, c0, c1, dflag = units[u]
                sync.dma_start(
                    out=K_s.ap()[4 * q : 4 * q + 4, c0:c1],
                    in_=stage_s[g % 4].ap()[
                        32 * s : 32 * s + 4, c0 - base : c1 - base
                    ],
                ).then_inc(ks, 16)

        # SP emits group slot-0/1 DMAs interleaved with C loads, ordered by
        # expected trigger time
        SP_AFTER = {10: [0, 1], 14: [2, 3], 18: [4, 5], 22: [6, 7],
                    26: [8], 30: [9]}

        @block.sync
        def _(sync):
            w = mk_waiter(sync)
            sync.dma_start(out=b2t_s.ap(), in_=b2t_d[:]).then_inc(dma_in, 16)
            for q in range(1, CR):
                sync.dma_start(
                    out=C_s[q].ap()[:, 0 : cwidth(q)], in_=C_d[q][:]
                ).then_inc(dma_in, 16)
            for s, d in [
                (w2t_s, w2t_d),
                (w3b_s, w3b_d),
                (mg_s[0], mg_d[0]),
                (mg_s[1], mg_d[1]),
                (dg_s[0], dg_d[0]),
                (dg_s[1], dg_d[1]),
                (m32_s, m32_d),
                (d32_s, d32_d),
            ]:
                sync.dma_start(out=s.ap(), in_=d[:]).then_inc(dma_in, 16)
            for q in range(CR, 32):
                w(acts, q - CR + 1)
                sync.dma_start(
                    out=C_s[q % CR].ap()[:, 0 : cwidth(q)], in_=C_d[q][:]
                ).then_inc(dma_in, 16)
                for g in SP_AFTER.get(q, []):
                    sp_group_dmas(sync, w, g)
            # tail-critical stage->K_s DMAs: groups 10 (s0,s1) + 11 (s0,s1)
            for g in (G - 2, G - 1):
                sp_group_dmas(sync, w, g)
            for k in range(4):
                w(evacv, k + 1)
                sync.dma_start(
                    out=oute_d[k][:], in_=o_s[k].ap()
                ).then_inc(odma, 16)
            for k in range(4, 12, 2):
                w(evacv, 5 + (k - 4) // 2)
                sync.dma_start(
                    out=outt_d[k - 4][:], in_=o_s[k].ap()
                ).then_inc(odma, 16)

        @block.scalar
        def _(scalar):
            w = mk_waiter(scalar)
            half = cwidth(0) // 2
            scalar.dma_start(
                out=C_s[0].ap()[:, 0:half], in_=C_d[0][:, 0:half]
            ).then_inc(c0s, 16)
            # trigger the Tanh table load while C0a streams in
            w(dma_in, 16)
            scalar.activation(
                t32_s.ap()[:, 0:1], b2t_s.ap(), Act.Tanh, bias=0.0, scale=1.0
            )
            scalar.dma_start(
                out=C_s[0].ap()[:, half : cwidth(0)],
                in_=C_d[0][:, half : cwidth(0)],
            ).then_inc(c0s, 16)
            for q in range(32):
                if q == 0:
                    w(c0s, 16)
                    scalar.activation(
                        hg_s[0].ap()[:, 0:half],
                        C_s[0].ap()[:, 0:half],
                        Act.Tanh,
                        bias=0.0,
                        scale=1.0,
                    )
                    w(c0s, 32)
                    scalar.activation(
                        hg_s[0].ap()[:, half : cwidth(0)],
                        C_s[0].ap()[:, half : cwidth(0)],
                        Act.Tanh,
                        bias=0.0,
                        scale=1.0,
                    ).then_inc(acts, 1)
                    continue
                w(dma_in, dmain_c(q))
                if q >= CR:
                    # hg/C ring: consumers of group q-CR done
                    lu = _u_last(q - CR)
                    w(pes, 5 * lu + 3)
                scalar.activation(
                    hg_s[q % CR].ap()[:, 0 : cwidth(q)],
                    C_s[q % CR].ap()[:, 0 : cwidth(q)],
                    Act.Tanh,
                    bias=0.0,
                    scale=1.0,
                ).then_inc(acts, 1)
            # tail: group 10/11 slots 2,3 DMAs from the scalar queue
            for g in (G - 2, G - 1):
                w(vex, g + 1)
                base, us = groups[g]
                for s in (2, 3):
                    u = us[s]
                    q, c0, c1, dflag = units[u]
                    scalar.dma_start(
                        out=K_s.ap()[4 * q : 4 * q + 4, c0:c1],
                        in_=stage_s[g % 4].ap()[
                            32 * s : 32 * s + 4, c0 - base : c1 - base
                        ],
                    ).then_inc(ks, 16)
            # odd tail tiles: evac on ACT, dma out on scalar queue
            for k in range(5, 12, 2):
                w(pek, k + 1)
                scalar.activation(
                    o_s[k].ap(),
                    pk_ps[k % 2].ap(),
                    Act.Copy,
                    bias=0.0,
                    scale=1.0,
                ).then_inc(evace, 1)
                scalar.dma_start(
                    out=outt_d[k - 4][:], in_=o_s[k].ap()
                ).then_inc(odma, 16)

        @block.tensor
        def _(tensor):
            w = mk_waiter(tensor)
            K_r = K_s.ap()
            w(dma_in, 16 * NPRE)

            def w3_mm(u):
                q, c0, c1, dflag = units[u]
                g, s = ug[u]
                gbase = groups[g][0]
                w(relu_s, u + 1)
                if g >= 2:
                    w(vex, g - 1)
                tensor.matmul(
                    pv_ps[g % 2].ap()[
                        32 * s : 32 * s + 4, c0 - gbase : c1 - gbase
                    ],
                    w3b_s.ap(),
                    h2_s[u % 4].ap()[:, 0 : c1 - c0],
                    start=True,
                    stop=True,
                    tile_position=(0, 32 * s),
                ).then_inc(pes, 1)

            for u, (q, c0, c1, dflag) in enumerate(units):
                cl = c1 - c0
                W = N - 32 * q
                if u >= 4:
                    w(relu_s, u - 3)
                for r in range(4):
                    w(acts, q + 1)
                    off = r * W + (c0 - 32 * q)
                    tensor.matmul(
                        ph_ps[u % 4].ap()[32 * r : 32 * r + 32, 0:cl],
                        w2t_s.ap(),
                        hg_s[q % CR].ap()[:, off : off + cl],
                        start=True,
                        stop=True,
                        tile_position=(0, 32 * r),
                    ).then_inc(pes, 1)
                if u >= 1:
                    w3_mm(u - 1)
                if u in EARLY_TILE_UNITS:
                    k = EARLY_TILE_UNITS[u]
                    # groups 0..6 cover all w0 rows; 2+2 dmas per group
                    w(kg, 32 * 7)
                    w(ks, 32 * 7)
                    if k >= 2:
                        w(evacv, k - 1)
                    tensor.matmul(
                        pk_ps[k % 2].ap(),
                        K_r[:, 128 * k : 128 * k + 128],
                        K_r[:, 0:512],
                        start=True,
                        stop=True,
                    ).then_inc(pek, 1)
            w3_mm(U - 1)
            # keep the PE clock ramped while waiting for the last K rows
            w(evacv, 4)
            for i, kc in enumerate((16 * 20, 16 * 22, 16 * 24, 16 * 26)):
                w(ks, kc)
                tensor.matmul(
                    pk_ps[i % 2].ap(),
                    K_r[:, 0:128],
                    K_r[:, 0:512],
                    start=True,
                    stop=True,
                )
            w(kg, 32 * 10)
            w(ks, 16 * 28)
            for k in range(4, 12):
                m = k - 4
                if k in (4, 5):
                    w(evacv, k - 1)
                elif k % 2 == 0:
                    w(evacv, 5 + (k - 6) // 2)
                else:
                    w(evace, 1 + (k - 7) // 2)
                tensor.matmul(
                    pk_ps[k % 2].ap(),
                    K_r[:, 128 * m : 128 * m + 128],
                    K_r[:, 512:1024],
                    start=True,
                    stop=True,
                ).then_inc(pek, 1)

        @block.vector
        def _(vector):
            w = mk_waiter(vector)

            def evac_group(g):
                lu = g_last_u[g]
                w(pes, 240 if lu == U - 1 else 5 * lu + 9)
                if g >= 4:
                    w(kg, 32 * (g - 3))
                    w(ks, 32 * (g - 3))
                if g >= G - 2:
                    # tail-critical groups: diag fixes fused via mask ops
                    i = g - (G - 2)
                    if b3v != 0.0:
                        vector.tensor_scalar(
                            stage_s[g % 4].ap(),
                            pv_ps[g % 2].ap(),
                            float(b3v),
                            None,
                            Alu.add,
                        )
                        vector.tensor_tensor(
                            stage_s[g % 4].ap(),
                            stage_s[g % 4].ap(),
                            mg_s[i].ap(),
                            Alu.mult,
                        )
                    else:
                        vector.tensor_tensor(
                            stage_s[g % 4].ap(),
                            pv_ps[g % 2].ap(),
                            mg_s[i].ap(),
                            Alu.mult,
                        )
                    vector.tensor_tensor(
                        stage_s[g % 4].ap(),
                        stage_s[g % 4].ap(),
                        dg_s[i].ap(),
                        Alu.add,
                    ).then_inc(vex, 1)
                elif b3v != 0.0:
                    vector.tensor_scalar(
                        stage_s[g % 4].ap(),
                        pv_ps[g % 2].ap(),
                        float(b3v),
                        None,
                        Alu.add,
                    ).then_inc(vex, 1)
                else:
                    vector.tensor_copy(
                        stage_s[g % 4].ap(), pv_ps[g % 2].ap()
                    ).then_inc(vex, 1)

            def evac_tile(k):
                w(pek, k + 1)
                vector.tensor_copy(o_s[k].ap(), pk_ps[k % 2].ap()).then_inc(
                    evacv, 1
                )

            w(dma_in, 16 * NPRE)
            pend = {
                g_last_u[g] + 1: g for g in range(G) if g_last_u[g] + 1 < U
            }
            for u, (q, c0, c1, dflag) in enumerate(units):
                cl = c1 - c0
                w(pes, 4 if u == 0 else 5 * u + 3)
                vector.tensor_scalar(
                    h2_s[u % 4].ap()[:, 0:cl],
                    ph_ps[u % 4].ap()[:, 0:cl],
                    b2t_s.ap(),
                    0.0,
                    Alu.add,
                    Alu.max,
                ).then_inc(relu_s, 1)
                if u in pend:
                    evac_group(pend[u])
                if u in EARLY_TILE_UNITS and EARLY_TILE_UNITS[u] >= 1:
                    evac_tile(EARLY_TILE_UNITS[u] - 1)
                if u == 44:
                    evac_tile(3)
            evac_group(G - 1)
            for k in range(4, 12, 2):
                evac_tile(k)

        @block.gpsimd
        def _(gpsimd):
            w = mk_waiter(gpsimd)
            gpsimd.memset(K_s.ap(), 0.0).then_inc(mset, 1)
            w(dma_in, 16 * NPRE)
            for g in range(G - 2):
                w(vex, g + 1)
                base, us = groups[g]
                if g in gp_fix_groups:
                    for s, u in enumerate(us):
                        q, c0, c1, dflag = units[u]
                        if not dflag:
                            continue
                        dwin = stage_s[g % 4].ap()[
                            32 * s : 32 * s + 4,
                            32 * q - base : 32 * q - base + 32,
                        ]
                        psl = slice(32 * s, 32 * s + 4)
                        gpsimd.tensor_tensor(
                            t32_s.ap()[psl, :], dwin, m32_s.ap()[psl, :],
                            Alu.mult,
                        )
                        gpsimd.tensor_tensor(
                            dwin, t32_s.ap()[psl, :], d32_s.ap()[psl, :],
                            Alu.add,
                        )
                for s in (2, 3):
                    u = us[s]
                    q, c0, c1, dflag = units[u]
                    gpsimd.dma_start(
                        out=K_s.ap()[4 * q : 4 * q + 4, c0:c1],
                        in_=stage_s[g % 4].ap()[
                            32 * s : 32 * s + 4, c0 - base : c1 - base
                        ],
                    ).then_inc(kg, 16)

    return nc


def kernel(x, W1, b1, W2, b2, W3, b3, sigma, _trace=False):
    from concourse.bass_utils import run_bass_kernel_spmd
    import jax.numpy as jnp

    x = np.asarray(x, np.float32).reshape(N)
    W1 = np.asarray(W1, np.float32)
    b1 = np.asarray(b1, np.float32).reshape(128)
    W2 = np.asarray(W2, np.float32)
    b2 = np.asarray(b2, np.float32).reshape(32)
    W3 = np.asarray(W3, np.float32).reshape(1, 32)
    b3v = float(np.asarray(b3, np.float32).reshape(-1)[0])
    sig = float(np.asarray(sigma, np.float32).reshape(-1)[0])

    key = b3v
    if key not in _BUILD_CACHE:
        _BUILD_CACHE[key] = _build(b3v)
    nc = _BUILD_CACHE[key]

    def to_bf16(a):
        return np.asarray(jnp.asarray(np.ascontiguousarray(a), jnp.bfloat16))

    B_f = (W1[:, 1:2] * x[None, :]).astype(np.float32)
    w2t_bf = to_bf16(W2.T)
    w3b = np.zeros((128, 4), np.float32)
    for g in range(4):
        w3b[32 * g : 32 * g + 32, g] = W3[0]
    w3b_bf = to_bf16(w3b)
    b2t = np.tile(b2.reshape(32, 1), (4, 1)).astype(np.float32)

    in_maps = []
    for c in range(NCORES):
        idx = 8 * np.arange(T) + c
        a_t = (W1[:, 0:1] * x[idx][None, :] + b1[:, None]).astype(np.float32)
        im = {
            "w2t": w2t_bf,
            "w3b": w3b_bf,
            "b2t": b2t,
        }
        allblocks = []
        bounds = [0]
        for q in range(32):
            for r in range(4):
                allblocks.append(
                    a_t[:, 4 * q + r : 4 * q + r + 1] + B_f[:, 32 * q :]
                )
            bounds.append(bounds[-1] + 4 * (N - 32 * q))
        packed = to_bf16(np.hstack(allblocks))
        for q in range(32):
            im[f"c{q}"] = np.ascontiguousarray(
                packed[:, bounds[q] : bounds[q + 1]]
            )
        m32 = np.zeros((128, 32), np.float32)
        d32 = np.zeros((128, 32), np.float32)
        for p in range(128):
            dc = 8 * (p % 4) + c
            m32[p, dc + 1 :] = 1.0
            d32[p, dc] = 1.0
        im["m32"] = to_bf16(m32)
        im["d32"] = to_bf16(d32)
        mgs, dgs = [], []
        for i in range(2):
            M = np.ones((128, 512), np.float32)
            D = np.zeros((128, 512), np.float32)
            for s in range(4):
                qs = 24 + 4 * i + s
                for r_ in range(4):
                    p = 32 * s + r_
                    dcol = 32 * qs - 512 + 8 * r_ + c
                    M[p, 32 * qs - 512 : dcol + 1] = 0.0
                    D[p, dcol] = 1.0
            mgs.append(to_bf16(M))
            dgs.append(to_bf16(D))
        im["mg0"], im["mg1"] = mgs
        im["dg0"], im["dg1"] = dgs
        in_maps.append(im)

    res = run_bass_kernel_spmd(
        nc, in_maps, core_ids=list(range(NCORES)), trace=_trace
    )
    global LAST_RESULT
    LAST_RESULT = res

    total = np.zeros((N, N), np.float32)
    for c in range(NCORES):
        r = res.results[c]
        for m in range(4):
            total[128 * m : 128 * m + 128, 0:512] += r[f"oe{m}"].astype(
                np.float32
            )
        for m in range(8):
            total[128 * m : 128 * m + 128, 512:1024] += r[f"ot{m}"].astype(
                np.float32
            )
    # G is symmetric: mirror the skipped lower-left block
    total[512:1024, 0:512] = total[0:512, 512:1024].T
    return (sig * sig) * total


# revision 45
# speedup vs baseline: 1.0242x; 1.0242x over previous
import sys

sys.path.insert(0, "/opt/trn_rl_repo")

import numpy as np

N = 1024
T = 128
NCORES = 8

_BUILD_CACHE = {}
LAST_RESULT = None


def _units():
    # Units aligned to absolute 512-col windows.
    # q < 16: two units [32q, 512) and [512, 1024); q >= 16: one [32q, 1024).
    units = []
    for q in range(32):
        if q < 16:
            units.append((q, 32 * q, 512, True))
            units.append((q, 512, 1024, False))
        else:
            units.append((q, 32 * q, 1024, True))
    return units


def _groups():
    # Each group: 4 units sharing one PSUM bank (partition slots 0,32,64,96),
    # one 512-col window. Returns list of (base_col, [unit_idx x4]).
    gs = []
    for k in range(4):
        gs.append((0, [2 * q for q in range(4 * k, 4 * k + 4)]))
        gs.append((512, [2 * q + 1 for q in range(4 * k, 4 * k + 4)]))
    for j in range(4):
        gs.append((512, [16 + q for q in range(16 + 4 * j, 16 + 4 * j + 4)]))
    return gs


def _u_last(q):
    return 2 * q + 1 if q < 16 else 16 + q


CR = 6  # C/h1-group ring depth (in q-groups)
EARLY_TILE_UNITS = {36: 0, 38: 1, 40: 2, 42: 3}  # unit -> early tile idx


def _build(b3v):
    import concourse.bass as bass
    from concourse import mybir

    F32 = mybir.dt.float32
    F16 = mybir.dt.float16
    BF16 = mybir.dt.bfloat16
    Alu = mybir.AluOpType
    Act = mybir.ActivationFunctionType

    units = _units()
    U = len(units)  # 48
    groups = _groups()
    G = len(groups)  # 12
    ug = {}
    for g, (base, us) in enumerate(groups):
        for s, u in enumerate(us):
            ug[u] = (g, s)
    g_last_u = [us[-1] for (base, us) in groups]
    # groups whose diag fixes run on gpsimd (mask-evac covers 10, 11)
    gp_fix_groups = [
        g for g in range(G - 2) if any(units[u][3] for u in groups[g][1])
    ]

    nc = bass.Bass("TRN2", target_bir_lowering=False, debug=False, num_devices=8)

    def cwidth(q):
        return 4 * (N - 32 * q)

    # ---- inputs ----
    # C_q = pre-biased tanh inputs for group q: [128, 4*(N-32q)] bf16,
    # laid out as 4 consecutive blocks (one per t=4q+r).
    C_d = [
        nc.dram_tensor(f"c{q}", (128, cwidth(q)), BF16, kind="ExternalInput")
        for q in range(32)
    ]
    w2t_d = nc.dram_tensor("w2t", (128, 32), BF16, kind="ExternalInput")
    w3b_d = nc.dram_tensor("w3b", (128, 4), BF16, kind="ExternalInput")
    b2t_d = nc.dram_tensor("b2t", (128, 1), F32, kind="ExternalInput")
    mg_d = [
        nc.dram_tensor(f"mg{i}", (128, 512), BF16, kind="ExternalInput")
        for i in range(2)
    ]
    dg_d = [
        nc.dram_tensor(f"dg{i}", (128, 512), BF16, kind="ExternalInput")
        for i in range(2)
    ]
    m32_d = nc.dram_tensor("m32", (128, 32), BF16, kind="ExternalInput")
    d32_d = nc.dram_tensor("d32", (128, 32), BF16, kind="ExternalInput")
    # outputs: oe{m} = G[128m:128m+128, 0:512] (m=0..3),
    # ot{m} = G[128m:128m+128, 512:1024] (m=0..7); f16 partials.
    oute_d = [
        nc.dram_tensor(f"oe{m}", (128, 512), F16, kind="ExternalOutput")
        for m in range(4)
    ]
    outt_d = [
        nc.dram_tensor(f"ot{m}", (128, 512), F16, kind="ExternalOutput")
        for m in range(8)
    ]

    # ---- sbuf ----
    C_s = [
        nc.alloc_sbuf_tensor(f"C_s{i}", [128, 4 * N], BF16) for i in range(CR)
    ]
    hg_s = [
        nc.alloc_sbuf_tensor(f"hg_s{i}", [128, 4 * N], BF16) for i in range(CR)
    ]
    w2t_s = nc.alloc_sbuf_tensor("w2t_s", [128, 32], BF16)
    w3b_s = nc.alloc_sbuf_tensor("w3b_s", [128, 4], BF16)
    b2t_s = nc.alloc_sbuf_tensor("b2t_s", [128, 1], F32)
    mg_s = [
        nc.alloc_sbuf_tensor(f"mg_s{i}", [128, 512], BF16) for i in range(2)
    ]
    dg_s = [
        nc.alloc_sbuf_tensor(f"dg_s{i}", [128, 512], BF16) for i in range(2)
    ]
    m32_s = nc.alloc_sbuf_tensor("m32_s", [128, 32], BF16)
    d32_s = nc.alloc_sbuf_tensor("d32_s", [128, 32], BF16)
    t32_s = nc.alloc_sbuf_tensor("t32_s", [128, 32], BF16)
    h2_s = [nc.alloc_sbuf_tensor(f"h2_{i}", [128, 512], BF16) for i in range(4)]
    stage_s = [
        nc.alloc_sbuf_tensor(f"stage{i}", [128, 512], BF16) for i in range(4)
    ]
    K_s = nc.alloc_sbuf_tensor("K_s", [128, N], BF16)
    o_s = [nc.alloc_sbuf_tensor(f"o_s{i}", [128, 512], F16) for i in range(12)]

    # ---- psum: 4 ph + 2 pv + 2 pk = 8 banks ----
    ph_ps = [nc.alloc_psum_tensor(f"ph{i}", [128, 512], F32) for i in range(4)]
    pv_ps = [nc.alloc_psum_tensor(f"pv{i}", [128, 512], F32) for i in range(2)]
    pk_ps = [nc.alloc_psum_tensor(f"pk{i}", [128, 512], F32) for i in range(2)]

    NPRE = CR + 10  # preloads: b2t, C0a, C0b, C1..C{CR-1}, 8 smalls

    def dmain_c(q):
        # dma_in value once C_q has landed
        if q == 0:
            return 48
        if q < CR:
            return 16 * (q + 3)
        return 16 * (NPRE + (q - CR) + 1)

    def mk_waiter(engine):
        seen = {}

        def w(sem, val):
            if seen.get(id(sem), 0) < val:
                engine.wait_ge(sem, val)
                seen[id(sem)] = val

        return w

    with (
        nc.Block() as block,
        nc.semaphore("dma_in") as dma_in,
        nc.semaphore("mset") as mset,
        nc.semaphore("acts") as acts,
        nc.semaphore("pes") as pes,
        nc.semaphore("pek") as pek,
        nc.semaphore("relu_s") as relu_s,
        nc.semaphore("vex") as vex,
        nc.semaphore("evacv") as evacv,
        nc.semaphore("evace") as evace,
        nc.semaphore("kg") as kg,
        nc.semaphore("ks") as ks,
        nc.semaphore("odma") as odma,
    ):
        def sp_group_dmas(sync, w, g):
            w(vex, g + 1)
            base, us = groups[g]
            for s in (0, 1):
                u = us[s]
                q, c0, c1, dflag = units[u]
                sync.dma_start(
                    out=K_s.ap()[4 * q : 4 * q + 4, c0:c1],
                    in_=stage_s[g % 4].ap()[
                        32 * s : 32 * s + 4, c0 - base : c1 - base
                    ],
                ).then_inc(ks, 16)

        # SP emits group slot-0/1 DMAs interleaved with C loads, ordered by
        # expected trigger time
        SP_AFTER = {10: [0, 1], 14: [2, 3], 18: [4, 5], 22: [6, 7],
                    26: [8], 30: [9]}

        @block.sync
        def _(sync):
            w = mk_waiter(sync)
            sync.dma_start(out=b2t_s.ap(), in_=b2t_d[:]).then_inc(dma_in, 16)
            half = cwidth(0) // 2
            sync.dma_start(
                out=C_s[0].ap()[:, 0:half], in_=C_d[0][:, 0:half]
            ).then_inc(dma_in, 16)
            sync.dma_start(
                out=C_s[0].ap()[:, half : cwidth(0)],
                in_=C_d[0][:, half : cwidth(0)],
            ).then_inc(dma_in, 16)
            for q in range(1, CR):
                sync.dma_start(
                    out=C_s[q].ap()[:, 0 : cwidth(q)], in_=C_d[q][:]
                ).then_inc(dma_in, 16)
            for s, d in [
                (w2t_s, w2t_d),
                (w3b_s, w3b_d),
                (mg_s[0], mg_d[0]),
                (mg_s[1], mg_d[1]),
                (dg_s[0], dg_d[0]),
                (dg_s[1], dg_d[1]),
                (m32_s, m32_d),
                (d32_s, d32_d),
            ]:
                sync.dma_start(out=s.ap(), in_=d[:]).then_inc(dma_in, 16)
            for q in range(CR, 32):
                w(acts, q - CR + 1)
                sync.dma_start(
                    out=C_s[q % CR].ap()[:, 0 : cwidth(q)], in_=C_d[q][:]
                ).then_inc(dma_in, 16)
                for g in SP_AFTER.get(q, []):
                    sp_group_dmas(sync, w, g)
            # tail-critical stage->K_s DMAs: groups 10 (s0,s1) + 11 (s0,s1)
            for g in (G - 2, G - 1):
                sp_group_dmas(sync, w, g)
            for k in range(4):
                w(evacv, k + 1)
                sync.dma_start(
                    out=oute_d[k][:], in_=o_s[k].ap()
                ).then_inc(odma, 16)
            for k in range(4, 12, 2):
                w(evacv, 5 + (k - 4) // 2)
                sync.dma_start(
                    out=outt_d[k - 4][:], in_=o_s[k].ap()
                ).then_inc(odma, 16)

        @block.scalar
        def _(scalar):
            w = mk_waiter(scalar)
            # trigger the Tanh table load before C0 lands
            w(dma_in, 16)
            scalar.activation(
                t32_s.ap()[:, 0:1], b2t_s.ap(), Act.Tanh, bias=0.0, scale=1.0
            )
            for q in range(32):
                if q == 0:
                    half = cwidth(0) // 2
                    w(dma_in, 32)
                    scalar.activation(
                        hg_s[0].ap()[:, 0:half],
                        C_s[0].ap()[:, 0:half],
                        Act.Tanh,
                        bias=0.0,
                        scale=1.0,
                    )
                    w(dma_in, 48)
                    scalar.activation(
                        hg_s[0].ap()[:, half : cwidth(0)],
                        C_s[0].ap()[:, half : cwidth(0)],
                        Act.Tanh,
                        bias=0.0,
                        scale=1.0,
                    ).then_inc(acts, 1)
                    continue
                w(dma_in, dmain_c(q))
                if q >= CR:
                    # hg/C ring: consumers of group q-CR done
                    lu = _u_last(q - CR)
                    w(pes, 5 * lu + 3)
                scalar.activation(
                    hg_s[q % CR].ap()[:, 0 : cwidth(q)],
                    C_s[q % CR].ap()[:, 0 : cwidth(q)],
                    Act.Tanh,
                    bias=0.0,
                    scale=1.0,
                ).then_inc(acts, 1)
            # tail: group 10/11 slots 2,3 DMAs from the scalar queue
            for g in (G - 2, G - 1):
                w(vex, g + 1)
                base, us = groups[g]
                for s in (2, 3):
                    u = us[s]
                    q, c0, c1, dflag = units[u]
                    scalar.dma_start(
                        out=K_s.ap()[4 * q : 4 * q + 4, c0:c1],
                        in_=stage_s[g % 4].ap()[
                            32 * s : 32 * s + 4, c0 - base : c1 - base
                        ],
                    ).then_inc(ks, 16)
            # odd tail tiles: evac on ACT, dma out on scalar queue
            for k in range(5, 12, 2):
                w(pek, k + 1)
                scalar.activation(
                    o_s[k].ap(),
                    pk_ps[k % 2].ap(),
                    Act.Copy,
                    bias=0.0,
                    scale=1.0,
                ).then_inc(evace, 1)
                scalar.dma_start(
                    out=outt_d[k - 4][:], in_=o_s[k].ap()
                ).then_inc(odma, 16)

        @block.tensor
        def _(tensor):
            w = mk_waiter(tensor)
            K_r = K_s.ap()
            w(dma_in, 16 * NPRE)

            def w3_mm(u):
                q, c0, c1, dflag = units[u]
                g, s = ug[u]
                gbase = groups[g][0]
                w(relu_s, u + 1)
                if g >= 2:
                    w(vex, g - 1)
                tensor.matmul(
                    pv_ps[g % 2].ap()[
                        32 * s : 32 * s + 4, c0 - gbase : c1 - gbase
                    ],
                    w3b_s.ap(),
                    h2_s[u % 4].ap()[:, 0 : c1 - c0],
                    start=True,
                    stop=True,
                    tile_position=(0, 32 * s),
                ).then_inc(pes, 1)

            for u, (q, c0, c1, dflag) in enumerate(units):
                cl = c1 - c0
                W = N - 32 * q
                if u >= 4:
                    w(relu_s, u - 3)
                for r in range(4):
                    w(acts, q + 1)
                    off = r * W + (c0 - 32 * q)
                    tensor.matmul(
                        ph_ps[u % 4].ap()[32 * r : 32 * r + 32, 0:cl],
                        w2t_s.ap(),
                        hg_s[q % CR].ap()[:, off : off + cl],
                        start=True,
                        stop=True,
                        tile_position=(0, 32 * r),
                    ).then_inc(pes, 1)
                if u >= 1:
                    w3_mm(u - 1)
                if u in EARLY_TILE_UNITS:
                    k = EARLY_TILE_UNITS[u]
                    # groups 0..6 cover all w0 rows; 2+2 dmas per group
                    w(kg, 32 * 7)
                    w(ks, 32 * 7)
                    if k >= 2:
                        w(evacv, k - 1)
                    tensor.matmul(
                        pk_ps[k % 2].ap(),
                        K_r[:, 128 * k : 128 * k + 128],
                        K_r[:, 0:512],
                        start=True,
                        stop=True,
                    ).then_inc(pek, 1)
            w3_mm(U - 1)
            # keep the PE clock ramped while waiting for the last K rows
            w(evacv, 4)
            for i, kc in enumerate((16 * 20, 16 * 22, 16 * 24, 16 * 26)):
                w(ks, kc)
                tensor.matmul(
                    pk_ps[i % 2].ap(),
                    K_r[:, 0:128],
                    K_r[:, 0:512],
                    start=True,
                    stop=True,
                )
            w(kg, 32 * 10)
            w(ks, 16 * 28)
            for k in range(4, 12):
                m = k - 4
                if k in (4, 5):
                    w(evacv, k - 1)
                elif k % 2 == 0:
                    w(evacv, 5 + (k - 6) // 2)
                else:
                    w(evace, 1 + (k - 7) // 2)
                tensor.matmul(
                    pk_ps[k % 2].ap(),
                    K_r[:, 128 * m : 128 * m + 128],
                    K_r[:, 512:1024],
                    start=True,
                    stop=True,
                ).then_inc(pek, 1)

        @block.vector
        def _(vector):
            w = mk_waiter(vector)

            def evac_group(g):
                lu = g_last_u[g]
                w(pes, 240 if lu == U - 1 else 5 * lu + 9)
                if g >= 4:
                    w(kg, 32 * (g - 3))
                    w(ks, 32 * (g - 3))
                if g >= G - 2:
                    # tail-critical groups: diag fixes fused via mask ops
                    i = g - (G - 2)
                    if b3v != 0.0:
                        vector.tensor_scalar(
                            stage_s[g % 4].ap(),
                            pv_ps[g % 2].ap(),
                            float(b3v),
                            None,
                            Alu.add,
                        )
                        vector.tensor_tensor(
                            stage_s[g % 4].ap(),
                            stage_s[g % 4].ap(),
                            mg_s[i].ap(),
                            Alu.mult,
                        )
                    else:
                        vector.tensor_tensor(
                            stage_s[g % 4].ap(),
                            pv_ps[g % 2].ap(),
                            mg_s[i].ap(),
                            Alu.mult,
                        )
                    vector.tensor_tensor(
                        stage_s[g % 4].ap(),
                        stage_s[g % 4].ap(),
                        dg_s[i].ap(),
                        Alu.add,
                    ).then_inc(vex, 1)
                elif b3v != 0.0:
                    vector.tensor_scalar(
                        stage_s[g % 4].ap(),
                        pv_ps[g % 2].ap(),
                        float(b3v),
                        None,
                        Alu.add,
                    ).then_inc(vex, 1)
                else:
                    vector.tensor_copy(
                        stage_s[g % 4].ap(), pv_ps[g % 2].ap()
                    ).then_inc(vex, 1)

            def evac_tile(k):
                w(pek, k + 1)
                vector.tensor_copy(o_s[k].ap(), pk_ps[k % 2].ap()).then_inc(
                    evacv, 1
                )

            w(dma_in, 16 * NPRE)
            pend = {
                g_last_u[g] + 1: g for g in range(G) if g_last_u[g] + 1 < U
            }
            for u, (q, c0, c1, dflag) in enumerate(units):
                cl = c1 - c0
                w(pes, 4 if u == 0 else 5 * u + 3)
                vector.tensor_scalar(
                    h2_s[u % 4].ap()[:, 0:cl],
                    ph_ps[u % 4].ap()[:, 0:cl],
                    b2t_s.ap(),
                    0.0,
                    Alu.add,
                    Alu.max,
                ).then_inc(relu_s, 1)
                if u in pend:
                    evac_group(pend[u])
                if u in EARLY_TILE_UNITS and EARLY_TILE_UNITS[u] >= 1:
                    evac_tile(EARLY_TILE_UNITS[u] - 1)
                if u == 44:
                    evac_tile(3)
            evac_group(G - 1)
            for k in range(4, 12, 2):
                evac_tile(k)

        @block.gpsimd
        def _(gpsimd):
            w = mk_waiter(gpsimd)
            gpsimd.memset(K_s.ap(), 0.0).then_inc(mset, 1)
            w(dma_in, 16 * NPRE)
            for g in range(G - 2):
                w(vex, g + 1)
                base, us = groups[g]
                if g in gp_fix_groups:
                    for s, u in enumerate(us):
                        q, c0, c1, dflag = units[u]
                        if not dflag:
                            continue
                        dwin = stage_s[g % 4].ap()[
                            32 * s : 32 * s + 4,
                            32 * q - base : 32 * q - base + 32,
                        ]
                        psl = slice(32 * s, 32 * s + 4)
                        gpsimd.tensor_tensor(
                            t32_s.ap()[psl, :], dwin, m32_s.ap()[psl, :],
                            Alu.mult,
                        )
                        gpsimd.tensor_tensor(
                            dwin, t32_s.ap()[psl, :], d32_s.ap()[psl, :],
                            Alu.add,
                        )
                for s in (2, 3):
                    u = us[s]
                    q, c0, c1, dflag = units[u]
                    gpsimd.dma_start(
                        out=K_s.ap()[4 * q : 4 * q + 4, c0:c1],
                        in_=stage_s[g % 4].ap()[
                            32 * s : 32 * s + 4, c0 - base : c1 - base
                        ],
                    ).then_inc(kg, 16)

    return nc


def kernel(x, W1, b1, W2, b2, W3, b3, sigma, _trace=False):
    from concourse.bass_utils import run_bass_kernel_spmd
    import jax.numpy as jnp

    x = np.asarray(x, np.float32).reshape(N)
    W1 = np.asarray(W1, np.float32)
    b1 = np.asarray(b1, np.float32).reshape(128)
    W2 = np.asarray(W2, np.float32)
    b2 = np.asarray(b2, np.float32).reshape(32)
    W3 = np.asarray(W3, np.float32).reshape(1, 32)
    b3v = float(np.asarray(b3, np.float32).reshape(-1)[0])
    sig = float(np.asarray(sigma, np.float32).reshape(-1)[0])

    key = b3v
    if key not in _BUILD_CACHE:
        _BUILD_CACHE[key] = _build(b3v)
    nc = _BUILD_CACHE[key]

    def to_bf16(a):
        return np.asarray(jnp.asarray(np.ascontiguousarray(a), jnp.bfloat16))

    B_f = (W1[:, 1:2] * x[None, :]).astype(np.float32)
    w2t_bf = to_bf16(W2.T)
    w3b = np.zeros((128, 4), np.float32)
    for g in range(4):
        w3b[32 * g : 32 * g + 32, g] = W3[0]
    w3b_bf = to_bf16(w3b)
    b2t = np.tile(b2.reshape(32, 1), (4, 1)).astype(np.float32)

    in_maps = []
    for c in range(NCORES):
        idx = 8 * np.arange(T) + c
        a_t = (W1[:, 0:1] * x[idx][None, :] + b1[:, None]).astype(np.float32)
        im = {
            "w2t": w2t_bf,
            "w3b": w3b_bf,
            "b2t": b2t,
        }
        allblocks = []
        bounds = [0]
        for q in range(32):
            for r in range(4):
                allblocks.append(
                    a_t[:, 4 * q + r : 4 * q + r + 1] + B_f[:, 32 * q :]
                )
            bounds.append(bounds[-1] + 4 * (N - 32 * q))
        packed = to_bf16(np.hstack(allblocks))
        for q in range(32):
            im[f"c{q}"] = np.ascontiguousarray(
                packed[:, bounds[q] : bounds[q + 1]]
            )
        m32 = np.zeros((128, 32), np.float32)
        d32 = np.zeros((128, 32), np.float32)
        for p in range(128):
            dc = 8 * (p % 4) + c
            m32[p, dc + 1 :] = 1.0
            d32[p, dc] = 1.0
        im["m32"] = to_bf16(m32)
        im["d32"] = to_bf16(d32)
        mgs, dgs = [], []
        for i in range(2):
            M = np.ones((128, 512), np.float32)
            D = np.zeros((128, 512), np.float32)
            for s in range(4):
                qs = 24 + 4 * i + s
                for r_ in range(4):
                    p = 32 * s + r_
                    dcol = 32 * qs - 512 + 8 * r_ + c
                    M[p, 32 * qs - 512 : dcol + 1] = 0.0
                    D[p, dcol] = 1.0
            mgs.append(to_bf16(M))
            dgs.append(to_bf16(D))
        im["mg0"], im["mg1"] = mgs
        im["dg0"], im["dg1"] = dgs
        in_maps.append(im)

    res = run_bass_kernel_spmd(
        nc, in_maps, core_ids=list(range(NCORES)), trace=_trace
    )
    global LAST_RESULT
    LAST_RESULT = res

    total = np.zeros((N, N), np.float32)
    for c in range(NCORES):
        r = res.results[c]
        for m in range(4):
            total[128 * m : 128 * m + 128, 0:512] += r[f"oe{m}"].astype(
                np.float32
            )
        for m in range(8):
            total[128 * m : 128 * m + 128, 512:1024] += r[f"ot{m}"].astype(
                np.float32
            )
    # G is symmetric: mirror the skipped lower-left block
    total[512:1024, 0:512] = total[0:512, 512:1024].T
    return (sig * sig) * total


# revision 46
# speedup vs baseline: 1.0469x; 1.0221x over previous
import sys

sys.path.insert(0, "/opt/trn_rl_repo")

import numpy as np

N = 1024
T = 128
NCORES = 8

_BUILD_CACHE = {}
LAST_RESULT = None


def _units():
    # Units aligned to absolute 512-col windows.
    # q < 16: two units [32q, 512) and [512, 1024); q >= 16: one [32q, 1024).
    units = []
    for q in range(32):
        if q < 16:
            units.append((q, 32 * q, 512, True))
            units.append((q, 512, 1024, False))
        else:
            units.append((q, 32 * q, 1024, True))
    return units


def _groups():
    # Each group: 4 units sharing one PSUM bank (partition slots 0,32,64,96),
    # one 512-col window. Returns list of (base_col, [unit_idx x4]).
    gs = []
    for k in range(4):
        gs.append((0, [2 * q for q in range(4 * k, 4 * k + 4)]))
        gs.append((512, [2 * q + 1 for q in range(4 * k, 4 * k + 4)]))
    for j in range(4):
        gs.append((512, [16 + q for q in range(16 + 4 * j, 16 + 4 * j + 4)]))
    return gs


def _u_last(q):
    return 2 * q + 1 if q < 16 else 16 + q


CR = 6  # C/h1-group ring depth (in q-groups)
EARLY_TILE_UNITS = {36: 0, 38: 1, 40: 2, 42: 3}  # unit -> early tile idx


def _build(b3v):
    import concourse.bass as bass
    from concourse import mybir

    F32 = mybir.dt.float32
    F16 = mybir.dt.float16
    BF16 = mybir.dt.bfloat16
    Alu = mybir.AluOpType
    Act = mybir.ActivationFunctionType

    units = _units()
    U = len(units)  # 48
    groups = _groups()
    G = len(groups)  # 12
    ug = {}
    for g, (base, us) in enumerate(groups):
        for s, u in enumerate(us):
            ug[u] = (g, s)
    g_last_u = [us[-1] for (base, us) in groups]
    # groups whose diag fixes run on gpsimd (mask-evac covers 10, 11)
    gp_fix_groups = [
        g for g in range(G - 2) if any(units[u][3] for u in groups[g][1])
    ]

    nc = bass.Bass("TRN2", target_bir_lowering=False, debug=False, num_devices=8)

    def cwidth(q):
        return 4 * (N - 32 * q)

    # ---- inputs ----
    # C_q = pre-biased tanh inputs for group q: [128, 4*(N-32q)] bf16,
    # laid out as 4 consecutive blocks (one per t=4q+r).
    C_d = [
        nc.dram_tensor(f"c{q}", (128, cwidth(q)), BF16, kind="ExternalInput")
        for q in range(32)
    ]
    w2t_d = nc.dram_tensor("w2t", (128, 32), BF16, kind="ExternalInput")
    w3b_d = nc.dram_tensor("w3b", (128, 4), BF16, kind="ExternalInput")
    b2t_d = nc.dram_tensor("b2t", (128, 1), F32, kind="ExternalInput")
    mg_d = [
        nc.dram_tensor(f"mg{i}", (128, 512), BF16, kind="ExternalInput")
        for i in range(2)
    ]
    dg_d = [
        nc.dram_tensor(f"dg{i}", (128, 512), BF16, kind="ExternalInput")
        for i in range(2)
    ]
    m32_d = nc.dram_tensor("m32", (128, 32), BF16, kind="ExternalInput")
    d32_d = nc.dram_tensor("d32", (128, 32), BF16, kind="ExternalInput")
    # outputs: oe{m} = G[128m:128m+128, 0:512] (m=0..3),
    # ot{m} = G[128m:128m+128, 512:1024] (m=0..7); f16 partials.
    oute_d = [
        nc.dram_tensor(f"oe{m}", (128, 512), F16, kind="ExternalOutput")
        for m in range(4)
    ]
    outt_d = [
        nc.dram_tensor(f"ot{m}", (128, 512), F16, kind="ExternalOutput")
        for m in range(8)
    ]

    # ---- sbuf ----
    C_s = [
        nc.alloc_sbuf_tensor(f"C_s{i}", [128, 4 * N], BF16) for i in range(CR)
    ]
    hg_s = [
        nc.alloc_sbuf_tensor(f"hg_s{i}", [128, 4 * N], BF16) for i in range(CR)
    ]
    w2t_s = nc.alloc_sbuf_tensor("w2t_s", [128, 32], BF16)
    w3b_s = nc.alloc_sbuf_tensor("w3b_s", [128, 4], BF16)
    b2t_s = nc.alloc_sbuf_tensor("b2t_s", [128, 1], F32)
    mg_s = [
        nc.alloc_sbuf_tensor(f"mg_s{i}", [128, 512], BF16) for i in range(2)
    ]
    dg_s = [
        nc.alloc_sbuf_tensor(f"dg_s{i}", [128, 512], BF16) for i in range(2)
    ]
    m32_s = nc.alloc_sbuf_tensor("m32_s", [128, 32], BF16)
    d32_s = nc.alloc_sbuf_tensor("d32_s", [128, 32], BF16)
    t32_s = nc.alloc_sbuf_tensor("t32_s", [128, 32], BF16)
    h2_s = [nc.alloc_sbuf_tensor(f"h2_{i}", [128, 512], BF16) for i in range(4)]
    stage_s = [
        nc.alloc_sbuf_tensor(f"stage{i}", [128, 512], BF16) for i in range(4)
    ]
    K_s = nc.alloc_sbuf_tensor("K_s", [128, N], BF16)
    o_s = [nc.alloc_sbuf_tensor(f"o_s{i}", [128, 512], F16) for i in range(12)]

    # ---- psum: 4 ph + 2 pv + 2 pk = 8 banks ----
    ph_ps = [nc.alloc_psum_tensor(f"ph{i}", [128, 512], F32) for i in range(4)]
    pv_ps = [nc.alloc_psum_tensor(f"pv{i}", [128, 512], F32) for i in range(2)]
    pk_ps = [nc.alloc_psum_tensor(f"pk{i}", [128, 512], F32) for i in range(2)]

    NPRE = CR + 10  # preloads: b2t, C0a, C0b, C1..C{CR-1}, 8 smalls

    def dmain_c(q):
        # dma_in value once C_q has landed
        if q == 0:
            return 48
        if q < CR:
            return 16 * (q + 3)
        return 16 * (NPRE + (q - CR) + 1)

    def mk_waiter(engine):
        seen = {}

        def w(sem, val):
            if seen.get(id(sem), 0) < val:
                engine.wait_ge(sem, val)
                seen[id(sem)] = val

        return w

    with (
        nc.Block() as block,
        nc.semaphore("dma_in") as dma_in,
        nc.semaphore("mset") as mset,
        nc.semaphore("acts") as acts,
        nc.semaphore("pes") as pes,
        nc.semaphore("pek") as pek,
        nc.semaphore("relu_s") as relu_s,
        nc.semaphore("vex") as vex,
        nc.semaphore("evacv") as evacv,
        nc.semaphore("evace") as evace,
        nc.semaphore("kg") as kg,
        nc.semaphore("ks") as ks,
        nc.semaphore("odma") as odma,
    ):
        def sp_group_dmas(sync, w, g):
            w(vex, g + 1)
            base, us = groups[g]
            for s in (0, 1):
                u = us[s]
                q, c0, c1, dflag = units[u]
                sync.dma_start(
                    out=K_s.ap()[4 * q : 4 * q + 4, c0:c1],
                    in_=stage_s[g % 4].ap()[
                        32 * s : 32 * s + 4, c0 - base : c1 - base
                    ],
                ).then_inc(ks, 16)

        # SP emits group slot-0/1 DMAs interleaved with C loads, ordered by
        # expected trigger time
        SP_AFTER = {10: [0, 1], 14: [2, 3], 18: [4, 5], 22: [6, 7],
                    26: [8], 30: [9]}

        @block.sync
        def _(sync):
            w = mk_waiter(sync)
            sync.dma_start(out=b2t_s.ap(), in_=b2t_d[:]).then_inc(dma_in, 16)
            half = cwidth(0) // 2
            sync.dma_start(
                out=C_s[0].ap()[:, 0:half], in_=C_d[0][:, 0:half]
            ).then_inc(dma_in, 16)
            sync.dma_start(
                out=C_s[0].ap()[:, half : cwidth(0)],
                in_=C_d[0][:, half : cwidth(0)],
            ).then_inc(dma_in, 16)
            for q in range(1, CR):
                sync.dma_start(
                    out=C_s[q].ap()[:, 0 : cwidth(q)], in_=C_d[q][:]
                ).then_inc(dma_in, 16)
            for s, d in [
                (w2t_s, w2t_d),
                (w3b_s, w3b_d),
                (mg_s[0], mg_d[0]),
                (mg_s[1], mg_d[1]),
                (dg_s[0], dg_d[0]),
                (dg_s[1], dg_d[1]),
                (m32_s, m32_d),
                (d32_s, d32_d),
            ]:
                sync.dma_start(out=s.ap(), in_=d[:]).then_inc(dma_in, 16)
            for q in range(CR, 32):
                w(acts, q - CR + 1)
                sync.dma_start(
                    out=C_s[q % CR].ap()[:, 0 : cwidth(q)], in_=C_d[q][:]
                ).then_inc(dma_in, 16)
                for g in SP_AFTER.get(q, []):
                    sp_group_dmas(sync, w, g)
            # tail-critical stage->K_s DMAs: groups 10 (s0,s1) + 11 (s0,s1)
            for g in (G - 2, G - 1):
                sp_group_dmas(sync, w, g)
            for k in range(4):
                w(evacv, k + 1)
                sync.dma_start(
                    out=oute_d[k][:], in_=o_s[k].ap()
                ).then_inc(odma, 16)
            for k in range(4, 12, 2):
                w(evacv, 5 + (k - 4) // 2)
                sync.dma_start(
                    out=outt_d[k - 4][:], in_=o_s[k].ap()
                ).then_inc(odma, 16)

        @block.scalar
        def _(scalar):
            w = mk_waiter(scalar)
            # trigger the Tanh table load before C0 lands
            w(dma_in, 16)
            scalar.activation(
                t32_s.ap()[:, 0:1], b2t_s.ap(), Act.Tanh, bias=0.0, scale=1.0
            )
            for q in range(32):
                if q == 0:
                    half = cwidth(0) // 2
                    w(dma_in, 32)
                    scalar.activation(
                        hg_s[0].ap()[:, 0:half],
                        C_s[0].ap()[:, 0:half],
                        Act.Tanh,
                        bias=0.0,
                        scale=1.0,
                    )
                    w(dma_in, 48)
                    scalar.activation(
                        hg_s[0].ap()[:, half : cwidth(0)],
                        C_s[0].ap()[:, half : cwidth(0)],
                        Act.Tanh,
                        bias=0.0,
                        scale=1.0,
                    ).then_inc(acts, 1)
                    continue
                w(dma_in, dmain_c(q))
                if q >= CR:
                    # hg/C ring: consumers of group q-CR done
                    lu = _u_last(q - CR)
                    w(pes, 5 * lu + 3)
                scalar.activation(
                    hg_s[q % CR].ap()[:, 0 : cwidth(q)],
                    C_s[q % CR].ap()[:, 0 : cwidth(q)],
                    Act.Tanh,
                    bias=0.0,
                    scale=1.0,
                ).then_inc(acts, 1)
            # tail: group 10/11 slots 2,3 DMAs from the scalar queue
            for g in (G - 2, G - 1):
                w(vex, g + 1)
                base, us = groups[g]
                for s in (2, 3):
                    u = us[s]
                    q, c0, c1, dflag = units[u]
                    scalar.dma_start(
                        out=K_s.ap()[4 * q : 4 * q + 4, c0:c1],
                        in_=stage_s[g % 4].ap()[
                            32 * s : 32 * s + 4, c0 - base : c1 - base
                        ],
                    ).then_inc(ks, 16)
            # odd tail tiles: evac on ACT, dma out on scalar queue
            for k in range(5, 12, 2):
                w(pek, k + 1)
                scalar.activation(
                    o_s[k].ap(),
                    pk_ps[k % 2].ap(),
                    Act.Copy,
                    bias=0.0,
                    scale=1.0,
                ).then_inc(evace, 1)
                scalar.dma_start(
                    out=outt_d[k - 4][:], in_=o_s[k].ap()
                ).then_inc(odma, 16)

        @block.tensor
        def _(tensor):
            w = mk_waiter(tensor)
            K_r = K_s.ap()
            w(dma_in, 16 * NPRE)

            def w3_mm(u):
                q, c0, c1, dflag = units[u]
                g, s = ug[u]
                gbase = groups[g][0]
                w(relu_s, u + 1)
                if g >= 2:
                    w(vex, g - 1)
                tensor.matmul(
                    pv_ps[g % 2].ap()[
                        32 * s : 32 * s + 4, c0 - gbase : c1 - gbase
                    ],
                    w3b_s.ap(),
                    h2_s[u % 4].ap()[:, 0 : c1 - c0],
                    start=True,
                    stop=True,
                    tile_position=(0, 32 * s),
                ).then_inc(pes, 1)

            for u, (q, c0, c1, dflag) in enumerate(units):
                cl = c1 - c0
                W = N - 32 * q
                if u >= 4:
                    w(relu_s, u - 3)
                for r in range(4):
                    w(acts, q + 1)
                    off = r * W + (c0 - 32 * q)
                    tensor.matmul(
                        ph_ps[u % 4].ap()[32 * r : 32 * r + 32, 0:cl],
                        w2t_s.ap(),
                        hg_s[q % CR].ap()[:, off : off + cl],
                        start=True,
                        stop=True,
                        tile_position=(0, 32 * r),
                    ).then_inc(pes, 1)
                if u >= 1:
                    w3_mm(u - 1)
                if u in EARLY_TILE_UNITS:
                    k = EARLY_TILE_UNITS[u]
                    # groups 0..6 cover all w0 rows; 2+2 dmas per group
                    w(kg, 32 * 7)
                    w(ks, 32 * 7)
                    if k >= 2:
                        w(evacv, k - 1)
                    tensor.matmul(
                        pk_ps[k % 2].ap(),
                        K_r[:, 128 * k : 128 * k + 128],
                        K_r[:, 0:512],
                        start=True,
                        stop=True,
                    ).then_inc(pek, 1)
            w3_mm(U - 1)
            # keep the PE clock ramped while waiting for the last K rows:
            # dummies paced across the whole tail-DMA window
            w(evacv, 4)
            w(vex, 11)
            tensor.matmul(
                pk_ps[0].ap(), K_r[:, 0:128], K_r[:, 0:512],
                start=True, stop=True,
            )
            for i, kc in enumerate(
                (320, 336, 352, 368, 384, 400, 416)
            ):
                w(ks, 16 * kc // 16)
                tensor.matmul(
                    pk_ps[(i + 1) % 2].ap(),
                    K_r[:, 0:128],
                    K_r[:, 0:512],
                    start=True,
                    stop=True,
                )
            w(kg, 32 * 10)
            w(ks, 16 * 28)
            for k in range(4, 12):
                m = k - 4
                if k in (4, 5):
                    w(evacv, k - 1)
                elif k % 2 == 0:
                    w(evacv, 5 + (k - 6) // 2)
                else:
                    w(evace, 1 + (k - 7) // 2)
                tensor.matmul(
                    pk_ps[k % 2].ap(),
                    K_r[:, 128 * m : 128 * m + 128],
                    K_r[:, 512:1024],
                    start=True,
                    stop=True,
                ).then_inc(pek, 1)

        @block.vector
        def _(vector):
            w = mk_waiter(vector)

            def evac_group(g):
                lu = g_last_u[g]
                w(pes, 240 if lu == U - 1 else 5 * lu + 9)
                if g >= 4:
                    w(kg, 32 * (g - 3))
                    w(ks, 32 * (g - 3))
                if g >= G - 2:
                    # tail-critical groups: diag fixes fused via mask ops
                    i = g - (G - 2)
                    if b3v != 0.0:
                        vector.tensor_scalar(
                            stage_s[g % 4].ap(),
                            pv_ps[g % 2].ap(),
                            float(b3v),
                            None,
                            Alu.add,
                        )
                        vector.tensor_tensor(
                            stage_s[g % 4].ap(),
                            stage_s[g % 4].ap(),
                            mg_s[i].ap(),
                            Alu.mult,
                        )
                    else:
                        vector.tensor_tensor(
                            stage_s[g % 4].ap(),
                            pv_ps[g % 2].ap(),
                            mg_s[i].ap(),
                            Alu.mult,
                        )
                    vector.tensor_tensor(
                        stage_s[g % 4].ap(),
                        stage_s[g % 4].ap(),
                        dg_s[i].ap(),
                        Alu.add,
                    ).then_inc(vex, 1)
                elif b3v != 0.0:
                    vector.tensor_scalar(
                        stage_s[g % 4].ap(),
                        pv_ps[g % 2].ap(),
                        float(b3v),
                        None,
                        Alu.add,
                    ).then_inc(vex, 1)
                else:
                    vector.tensor_copy(
                        stage_s[g % 4].ap(), pv_ps[g % 2].ap()
                    ).then_inc(vex, 1)

            def evac_tile(k):
                w(pek, k + 1)
                vector.tensor_copy(o_s[k].ap(), pk_ps[k % 2].ap()).then_inc(
                    evacv, 1
                )

            w(dma_in, 16 * NPRE)
            pend = {
                g_last_u[g] + 1: g for g in range(G) if g_last_u[g] + 1 < U
            }
            for u, (q, c0, c1, dflag) in enumerate(units):
                cl = c1 - c0
                w(pes, 4 if u == 0 else 5 * u + 3)
                vector.tensor_scalar(
                    h2_s[u % 4].ap()[:, 0:cl],
                    ph_ps[u % 4].ap()[:, 0:cl],
                    b2t_s.ap(),
                    0.0,
                    Alu.add,
                    Alu.max,
                ).then_inc(relu_s, 1)
                if u in pend:
                    evac_group(pend[u])
                if u in EARLY_TILE_UNITS and EARLY_TILE_UNITS[u] >= 1:
                    evac_tile(EARLY_TILE_UNITS[u] - 1)
                if u == 44:
                    evac_tile(3)
            evac_group(G - 1)
            for k in range(4, 12, 2):
                evac_tile(k)

        @block.gpsimd
        def _(gpsimd):
            w = mk_waiter(gpsimd)
            gpsimd.memset(K_s.ap(), 0.0).then_inc(mset, 1)
            w(dma_in, 16 * NPRE)
            for g in range(G - 2):
                w(vex, g + 1)
                base, us = groups[g]
                if g in gp_fix_groups:
                    for s, u in enumerate(us):
                        q, c0, c1, dflag = units[u]
                        if not dflag:
                            continue
                        dwin = stage_s[g % 4].ap()[
                            32 * s : 32 * s + 4,
                            32 * q - base : 32 * q - base + 32,
                        ]
                        psl = slice(32 * s, 32 * s + 4)
                        gpsimd.tensor_tensor(
                            t32_s.ap()[psl, :], dwin, m32_s.ap()[psl, :],
                            Alu.mult,
                        )
                        gpsimd.tensor_tensor(
                            dwin, t32_s.ap()[psl, :], d32_s.ap()[psl, :],
                            Alu.add,
                        )
                for s in (2, 3):
                    u = us[s]
                    q, c0, c1, dflag = units[u]
                    gpsimd.dma_start(
                        out=K_s.ap()[4 * q : 4 * q + 4, c0:c1],
                        in_=stage_s[g % 4].ap()[
                            32 * s : 32 * s + 4, c0 - base : c1 - base
                        ],
                    ).then_inc(kg, 16)

    return nc


def kernel(x, W1, b1, W2, b2, W3, b3, sigma, _trace=False):
    from concourse.bass_utils import run_bass_kernel_spmd
    import jax.numpy as jnp

    x = np.asarray(x, np.float32).reshape(N)
    W1 = np.asarray(W1, np.float32)
    b1 = np.asarray(b1, np.float32).reshape(128)
    W2 = np.asarray(W2, np.float32)
    b2 = np.asarray(b2, np.float32).reshape(32)
    W3 = np.asarray(W3, np.float32).reshape(1, 32)
    b3v = float(np.asarray(b3, np.float32).reshape(-1)[0])
    sig = float(np.asarray(sigma, np.float32).reshape(-1)[0])

    key = b3v
    if key not in _BUILD_CACHE:
        _BUILD_CACHE[key] = _build(b3v)
    nc = _BUILD_CACHE[key]

    def to_bf16(a):
        return np.asarray(jnp.asarray(np.ascontiguousarray(a), jnp.bfloat16))

    B_f = (W1[:, 1:2] * x[None, :]).astype(np.float32)
    w2t_bf = to_bf16(W2.T)
    w3b = np.zeros((128, 4), np.float32)
    for g in range(4):
        w3b[32 * g : 32 * g + 32, g] = W3[0]
    w3b_bf = to_bf16(w3b)
    b2t = np.tile(b2.reshape(32, 1), (4, 1)).astype(np.float32)

    in_maps = []
    for c in range(NCORES):
        idx = 8 * np.arange(T) + c
        a_t = (W1[:, 0:1] * x[idx][None, :] + b1[:, None]).astype(np.float32)
        im = {
            "w2t": w2t_bf,
            "w3b": w3b_bf,
            "b2t": b2t,
        }
        allblocks = []
        bounds = [0]
        for q in range(32):
            for r in range(4):
                allblocks.append(
                    a_t[:, 4 * q + r : 4 * q + r + 1] + B_f[:, 32 * q :]
                )
            bounds.append(bounds[-1] + 4 * (N - 32 * q))
        packed = to_bf16(np.hstack(allblocks))
        for q in range(32):
            im[f"c{q}"] = np.ascontiguousarray(
                packed[:, bounds[q] : bounds[q + 1]]
            )
        m32 = np.zeros((128, 32), np.float32)
        d32 = np.zeros((128, 32), np.float32)
        for p in range(128):
            dc = 8 * (p % 4) + c
            m32[p, dc + 1 :] = 1.0
            d32[p, dc] = 1.0
        im["m32"] = to_bf16(m32)
        im["d32"] = to_bf16(d32)
        mgs, dgs = [], []
        for i in range(2):
            M = np.ones((128, 512), np.float32)
            D = np.zeros((128, 512), np.float32)
            for s in range(4):
                qs = 24 + 4 * i + s
                for r_ in range(4):
                    p = 32 * s + r_
                    dcol = 32 * qs - 512 + 8 * r_ + c
                    M[p, 32 * qs - 512 : dcol + 1] = 0.0
                    D[p, dcol] = 1.0
            mgs.append(to_bf16(M))
            dgs.append(to_bf16(D))
        im["mg0"], im["mg1"] = mgs
        im["dg0"], im["dg1"] = dgs
        in_maps.append(im)

    res = run_bass_kernel_spmd(
        nc, in_maps, core_ids=list(range(NCORES)), trace=_trace
    )
    global LAST_RESULT
    LAST_RESULT = res

    total = np.zeros((N, N), np.float32)
    for c in range(NCORES):
        r = res.results[c]
        for m in range(4):
            total[128 * m : 128 * m + 128, 0:512] += r[f"oe{m}"].astype(
                np.float32
            )
        for m in range(8):
            total[128 * m : 128 * m + 128, 512:1024] += r[f"ot{m}"].astype(
                np.float32
            )
    # G is symmetric: mirror the skipped lower-left block
    total[512:1024, 0:512] = total[0:512, 512:1024].T
    return (sig * sig) * total
